# revision 1
# baseline (speedup 1.0000x reference)
"""CrossTransformer Trainium2 kernel.

Shapes (hardcoded): B=4, TQ=TK=1024, D=1024, H=16, DK=DV=64.
Sharding: 8 cores = 4 batches x 2 query-row halves. Each core computes
out[b, qs:qs+512, :] independently (k/v work duplicated across the pair
of cores sharing a batch; no collectives).

Weights are converted to bf16 on the host (the matmuls are bf16 either
way; converting host-side halves the weight DMA traffic and removes 48
on-device cast ops).
"""
import sys

for _p in ("/root/.axon_site", "/root/.axon_site/_ro/trn_rl_repo",
           "/root/.axon_site/_ro/pypackages", "/opt/trn_rl_repo"):
    if _p not in sys.path:
        sys.path.append(_p)

import numpy as np
import ml_dtypes
import concourse.bass as bass
from concourse import bacc
import concourse.tile as tile
import concourse.mybir as mybir
from concourse.masks import make_identity
from concourse.bass_utils import run_bass_kernel_spmd

F32 = mybir.dt.float32
BF = mybir.dt.bfloat16
AF = mybir.ActivationFunctionType
OP = mybir.AluOpType

B, TQ, TK, D = 4, 1024, 1024, 1024
H, DK, DV = 16, 64, 64
TQC = TQ // 2          # 512 query rows per core
NT = TQC // 128        # 4 q-row tiles
KD = D // 128          # 8 contraction chunks
MASK_NEG = -30000.0

WEIGHT_NAMES = ["q_w", "k_w", "v_w", "o_w", "l1_w", "l2_w"]
VEC_NAMES = ["q_b", "k_b", "v_b", "o_b", "l1_b", "l2_b",
             "ln1_g", "ln1_b", "ln2_g", "ln2_b",
             "mln1_g", "mln1_b", "mln2_g", "mln2_b"]


def build_kernel(compile=True, repeat=1, stop_after=None):
    nc = bacc.Bacc()
    xs = nc.dram_tensor("xs", (TQC, D), F32, kind="ExternalInput")
    y = nc.dram_tensor("y", (TK, D), F32, kind="ExternalInput")
    mb = nc.dram_tensor("mb", (TK,), F32, kind="ExternalInput")
    wd = {n: nc.dram_tensor(n, (D, D), BF, kind="ExternalInput") for n in WEIGHT_NAMES}
    vd = {n: nc.dram_tensor(n, (D,), F32, kind="ExternalInput") for n in VEC_NAMES}
    out = nc.dram_tensor("out", (TQC, D), F32, kind="ExternalOutput")

    with tile.TileContext(nc) as tc:
        for r in range(repeat):
            _emit(nc, tc, xs, y, mb, wd, vd, out, pfx=f"r{r}_", stop_after=stop_after)
    if compile:
        nc.compile()
    return nc


def _emit(nc, tc, xs, y, mb, wd, vd, out, pfx="", stop_after=None):
    from contextlib import ExitStack

    ctx = ExitStack()
    with ctx:
        persist = ctx.enter_context(tc.tile_pool(name=pfx + "persist", bufs=1))
        lnp = ctx.enter_context(tc.tile_pool(name=pfx + "lnp", bufs=2))
        bcast = ctx.enter_context(tc.tile_pool(name=pfx + "bcast", bufs=4))
        wts = ctx.enter_context(tc.tile_pool(name=pfx + "wts", bufs=2))
        psmm = ctx.enter_context(tc.tile_pool(name=pfx + "psmm", bufs=4, space="PSUM"))
        psmi = ctx.enter_context(tc.tile_pool(name=pfx + "psmi", bufs=2, space="PSUM"))

        # ---------------- setup constants ----------------
        ident = persist.tile([128, 128], BF, tag="ident", name=pfx + "ident")
        make_identity(nc, ident)
        eps_t = persist.tile([128, 1], F32, tag="eps", name=pfx + "eps")
        nc.vector.memset(eps_t[:], 1e-5)
        ones_c = persist.tile([128, DV], BF, tag="ones_c", name=pfx + "ones_c")
        nc.vector.memset(ones_c[:], 1.0)
        mb_sb = persist.tile([128, KD], F32, tag="mb_sb", name=pfx + "mb_sb")
        nc.sync.dma_start(mb_sb[:], mb.rearrange("(t p) -> p t", p=128))
        bq_sb = persist.tile([128, KD], F32, tag="bq_sb", name=pfx + "bq_sb")
        nc.sync.dma_start(bq_sb[:], vd["q_b"].rearrange("(t p) -> p t", p=128))
        bk_sb = persist.tile([128, KD], F32, tag="bk_sb", name=pfx + "bk_sb")
        nc.sync.dma_start(bk_sb[:], vd["k_b"].rearrange("(t p) -> p t", p=128))

        def bcast_tile(name):
            t = bcast.tile([128, D], F32, tag="bc", name=pfx + f"bc_{name}")
            nc.sync.dma_start(t[:], vd[name][:].unsqueeze(0).partition_broadcast(128))
            return t

        def load_weight(name):
            wt = wts.tile([128, KD, D], BF, tag="wbf", name=pfx + f"wbf_{name}")
            nc.sync.dma_start(wt[:], wd[name].rearrange("(ko p) n -> p ko n", p=128))
            return wt

        # LN(+affine)+ReLU: src [128, D] f32 -> dst [128, D] bf16
        def ln_relu(src, gt, bt, dst, key):
            stats = lnp.tile([128, 2, 6], F32, tag="stats", name=pfx + f"st_{key}")
            for i in range(2):
                nc.vector.bn_stats(stats[:, i, :], src[:, i * 512:(i + 1) * 512])
            mv = lnp.tile([128, 2], F32, tag="mv", name=pfx + f"mv_{key}")
            nc.vector.bn_aggr(mv[:], stats[:])
            std = lnp.tile([128, 1], F32, tag="std", name=pfx + f"sd_{key}")
            nc.scalar.activation(std[:], mv[:, 1:2], AF.Sqrt, bias=eps_t[:], scale=1.0)
            rstd = lnp.tile([128, 1], F32, tag="rstd", name=pfx + f"rs_{key}")
            nc.vector.reciprocal(rstd[:], std[:])
            z = lnp.tile([128, D], F32, tag="lnz", name=pfx + f"z_{key}")
            nc.vector.tensor_scalar(z[:], src[:], mv[:, 0:1], rstd[:],
                                    OP.subtract, OP.mult)
            nc.gpsimd.tensor_tensor(z[:], z[:], gt[:], OP.mult)
            nc.gpsimd.tensor_tensor(z[:], z[:], bt[:], OP.add)
            nc.scalar.activation(dst, z[:], AF.Relu, scale=1.0)

        # PE transpose of a [128,128] bf16 block; psum->sbuf copy on DVE
        tr_count = [0]

        def transpose_128(src_ap, dst_ap):
            pt = psmi.tile([128, 128], BF, tag="ps_tr", name=pfx + f"tr{tr_count[0]}")
            nc.tensor.transpose(pt[:], src_ap, ident[:])
            nc.vector.tensor_copy(dst_ap, pt[:])
            tr_count[0] += 1

        x_sb = persist.tile([128, NT, D], F32, tag="x_sb", name=pfx + "x_sb")
        qT = persist.tile([128, KD, TQC], BF, tag="qT", name=pfx + "qT")
        kT = persist.tile([128, KD, TK], BF, tag="kT", name=pfx + "kT")
        v_ext = persist.tile([128, KD, H, DV + 1], BF, tag="v_ext", name=pfx + "v_ext")
        attnT = persist.tile([128, KD, TQC], BF, tag="attnT", name=pfx + "attnT")

        with (
            tc.tile_pool(name=pfx + "pab", bufs=1) as pab,
            tc.tile_pool(name=pfx + "ldp", bufs=2) as ldp,
            tc.tile_pool(name=pfx + "ptr", bufs=1) as ptr,
        ):
            # ---------------- phase A: LN + relu ----------------
            g1 = bcast_tile("ln1_g")
            b1 = bcast_tile("ln1_b")
            g2 = bcast_tile("ln2_g")
            b2 = bcast_tile("ln2_b")

            x1 = pab.tile([128, NT, D], BF, tag="x1", name=pfx + "x1")
            for t in range(NT):
                nc.sync.dma_start(x_sb[:, t, :],
                                  xs.rearrange("(t p) d -> p t d", p=128)[:, t, :])
                ln_relu(x_sb[:, t, :], g1, b1, x1[:, t, :], f"x{t}")

            y1 = pab.tile([128, KD, D], BF, tag="y1", name=pfx + "y1")
            for t in range(KD):
                yl = ldp.tile([128, D], F32, tag="yload", name=pfx + f"yl_{t}")
                nc.sync.dma_start(yl[:], y.rearrange("(t p) d -> p t d", p=128)[:, t, :])
                ln_relu(yl[:], g2, b2, y1[:, t, :], f"y{t}")

            if stop_after == "A":
                return
            # ---------------- phase B: transposes ----------------
            x1T = ptr.tile([128, KD, TQC], BF, tag="x1T", name=pfx + "x1T")
            for dt in range(KD):
                for tt in range(NT):
                    transpose_128(x1[:, tt, dt * 128:(dt + 1) * 128],
                                  x1T[:, dt, tt * 128:(tt + 1) * 128])
            y1T = ptr.tile([128, KD, TK], BF, tag="y1T", name=pfx + "y1T")
            for dt in range(KD):
                for tt in range(KD):
                    transpose_128(y1[:, tt, dt * 128:(dt + 1) * 128],
                                  y1T[:, dt, tt * 128:(tt + 1) * 128])

            if stop_after == "B":
                return
            # ---------------- phase C: projections ----------------
            # qT[hdk, tq] = q_w.T @ x1T
            wq = load_weight("q_w")
            for m in range(KD):
                pq = psmm.tile([128, TQC], F32, tag="ps_mm", name=pfx + f"pq{m}")
                for kc in range(KD):
                    nc.tensor.matmul(pq[:], wq[:, kc, m * 128:(m + 1) * 128],
                                     x1T[:, kc, :],
                                     start=(kc == 0), stop=(kc == KD - 1))
                nc.scalar.activation(qT[:, m, :], pq[:], AF.Identity,
                                     bias=bq_sb[:, m:m + 1], scale=1.0)

            # kT[hdk, tk] = k_w.T @ y1T
            wk = load_weight("k_w")
            for m in range(KD):
                for nt2 in range(2):
                    pk = psmm.tile([128, 512], F32, tag="ps_mm", name=pfx + f"pk{m}_{nt2}")
                    for kc in range(KD):
                        nc.tensor.matmul(pk[:], wk[:, kc, m * 128:(m + 1) * 128],
                                         y1T[:, kc, nt2 * 512:(nt2 + 1) * 512],
                                         start=(kc == 0), stop=(kc == KD - 1))
                    nc.scalar.activation(kT[:, m, nt2 * 512:(nt2 + 1) * 512], pk[:],
                                         AF.Identity, bias=bk_sb[:, m:m + 1], scale=1.0)

            # v[tk, hdv] (+ones col) = y1 @ v_w
            wv = load_weight("v_w")
            bv = bcast_tile("v_b")
            nc.vector.memset(v_ext[:, :, :, DV:], 1.0)
            for m in range(KD):
                for nt2 in range(2):
                    pv = psmm.tile([128, 512], F32, tag="ps_mm", name=pfx + f"pv{m}_{nt2}")
                    for kc in range(KD):
                        nc.tensor.matmul(pv[:], y1T[:, kc, m * 128:(m + 1) * 128],
                                         wv[:, kc, nt2 * 512:(nt2 + 1) * 512],
                                         start=(kc == 0), stop=(kc == KD - 1))
                    nc.vector.tensor_tensor(
                        v_ext[:, m, nt2 * 8:(nt2 + 1) * 8, :DV],
                        pv.rearrange("p (h v) -> p h v", v=DV),
                        bv[:, nt2 * 512:(nt2 + 1) * 512].rearrange(
                            "p (h v) -> p h v", v=DV),
                        OP.add)

        if stop_after == "C":
            return
        # ---------------- phase D: attention (head pairs) ----------------
        with tc.tile_pool(name=pfx + "att", bufs=2) as att:
            for j in range(KD):      # head pair j -> heads 2j (rows 0:64), 2j+1 (64:128)
                e_sb = att.tile([128, 2, KD, TQC], BF, tag="e_sb", name=pfx + f"e{j}")
                for mt in range(KD):
                    ps0 = psmm.tile([128, TQC], F32, tag="ps_mm", name=pfx + f"s{j}_{mt}a")
                    ps1 = psmm.tile([128, TQC], F32, tag="ps_mm", name=pfx + f"s{j}_{mt}b")
                    # row-tiled pair: K=64 each, concurrent on PE row groups
                    nc.tensor.matmul(ps0[:], kT[0:64, j, mt * 128:(mt + 1) * 128],
                                     qT[0:64, j, :], start=True, stop=True)
                    nc.tensor.matmul(ps1[:], kT[64:128, j, mt * 128:(mt + 1) * 128],
                                     qT[64:128, j, :], start=True, stop=True)
                    nc.scalar.activation(e_sb[:, 0, mt, :], ps0[:], AF.Exp,
                                         bias=mb_sb[:, mt:mt + 1], scale=0.125)
                    nc.scalar.activation(e_sb[:, 1, mt, :], ps1[:], AF.Exp,
                                         bias=mb_sb[:, mt:mt + 1], scale=0.125)
                for par in range(2):
                    h = 2 * j + par
                    oh = par * 64
                    ps_av = psmi.tile([128, TQC], F32, tag="ps_av", name=pfx + f"av{h}")
                    for kt in range(KD):
                        nc.tensor.matmul(ps_av[:DV + 1, :], v_ext[:, kt, h, :],
                                         e_sb[:, par, kt, :],
                                         start=(kt == 0), stop=(kt == KD - 1))
                    rcp = att.tile([128, TQC], F32, tag="rcp", name=pfx + f"rc{h}")
                    nc.vector.reciprocal(rcp[DV:DV + 1, :], ps_av[DV:DV + 1, :])
                    rcb = att.tile([128, TQC], BF, tag="rcb", name=pfx + f"rb{h}")
                    nc.vector.tensor_copy(rcb[DV:DV + 1, :], rcp[DV:DV + 1, :])
                    ps_bc = psmi.tile([DV, TQC], F32, tag="ps_tr", name=pfx + f"bc{h}")
                    nc.tensor.matmul(ps_bc[:], ones_c[DV:DV + 1, :],
                                     rcb[DV:DV + 1, :], start=True, stop=True)
                    rb_sb = att.tile([DV, TQC], F32, tag="rb_sb", name=pfx + f"rs{h}")
                    nc.scalar.activation(rb_sb[:], ps_bc[:], AF.Identity, scale=1.0)
                    nc.vector.tensor_tensor(attnT[oh:oh + DV, j, :], ps_av[:DV, :],
                                            rb_sb[:], OP.mult)

        if stop_after == "D":
            return
        # ---------------- phase E: o-proj + residual ----------------
        wo = load_weight("o_w")
        bo = bcast_tile("o_b")
        for mt in range(NT):
            for nt2 in range(2):
                po = psmm.tile([128, 512], F32, tag="ps_mm", name=pfx + f"po{mt}_{nt2}")
                for kc in range(KD):
                    nc.tensor.matmul(po[:], attnT[:, kc, mt * 128:(mt + 1) * 128],
                                     wo[:, kc, nt2 * 512:(nt2 + 1) * 512],
                                     start=(kc == 0), stop=(kc == KD - 1))
                sl = slice(nt2 * 512, (nt2 + 1) * 512)
                nc.vector.tensor_tensor(x_sb[:, mt, sl], x_sb[:, mt, sl], po[:], OP.add)
                nc.gpsimd.tensor_tensor(x_sb[:, mt, sl], x_sb[:, mt, sl], bo[:, sl], OP.add)

        if stop_after == "E":
            return
        # ---------------- phases F/G: MLP ----------------
        with (
            tc.tile_pool(name=pfx + "mlp", bufs=1) as mlp,
            tc.tile_pool(name=pfx + "mtr", bufs=1) as mtr,
        ):
            g3 = bcast_tile("mln1_g")
            b3 = bcast_tile("mln1_b")
            z1 = mlp.tile([128, NT, D], BF, tag="z1", name=pfx + "z1")
            for t in range(NT):
                ln_relu(x_sb[:, t, :], g3, b3, z1[:, t, :], f"z1_{t}")
            z1T = mtr.tile([128, KD, TQC], BF, tag="z1T", name=pfx + "z1T")
            for dt in range(KD):
                for tt in range(NT):
                    transpose_128(z1[:, tt, dt * 128:(dt + 1) * 128],
                                  z1T[:, dt, tt * 128:(tt + 1) * 128])
            w1 = load_weight("l1_w")
            bl1 = bcast_tile("l1_b")
            h_sb = mlp.tile([128, NT, D], F32, tag="h_sb", name=pfx + "h_sb")
            for mt in range(NT):
                for nt2 in range(2):
                    ph = psmm.tile([128, 512], F32, tag="ps_mm", name=pfx + f"ph{mt}_{nt2}")
                    for kc in range(KD):
                        nc.tensor.matmul(ph[:], z1T[:, kc, mt * 128:(mt + 1) * 128],
                                         w1[:, kc, nt2 * 512:(nt2 + 1) * 512],
                                         start=(kc == 0), stop=(kc == KD - 1))
                    sl = slice(nt2 * 512, (nt2 + 1) * 512)
                    nc.vector.tensor_tensor(h_sb[:, mt, sl], ph[:], bl1[:, sl], OP.add)

            g4 = bcast_tile("mln2_g")
            b4 = bcast_tile("mln2_b")
            z2 = mlp.tile([128, NT, D], BF, tag="z2", name=pfx + "z2")
            for t in range(NT):
                ln_relu(h_sb[:, t, :], g4, b4, z2[:, t, :], f"z2_{t}")
            z2T = mtr.tile([128, KD, TQC], BF, tag="z2T", name=pfx + "z2T")
            for dt in range(KD):
                for tt in range(NT):
                    transpose_128(z2[:, tt, dt * 128:(dt + 1) * 128],
                                  z2T[:, dt, tt * 128:(tt + 1) * 128])
            w2 = load_weight("l2_w")
            bl2 = bcast_tile("l2_b")
            out_r = out.rearrange("(t p) d -> p t d", p=128)
            for mt in range(NT):
                o_sb = mlp.tile([128, D], F32, tag="o_sb", name=pfx + f"os{mt}")
                for nt2 in range(2):
                    pf = psmm.tile([128, 512], F32, tag="ps_mm", name=pfx + f"pf{mt}_{nt2}")
                    for kc in range(KD):
                        nc.tensor.matmul(pf[:], z2T[:, kc, mt * 128:(mt + 1) * 128],
                                         w2[:, kc, nt2 * 512:(nt2 + 1) * 512],
                                         start=(kc == 0), stop=(kc == KD - 1))
                    sl = slice(nt2 * 512, (nt2 + 1) * 512)
                    nc.vector.tensor_tensor(o_sb[:, sl], pf[:], bl2[:, sl], OP.add)
                nc.sync.dma_start(out_r[:, mt, :], o_sb[:])


_NC_CACHE = None


def _get_nc():
    global _NC_CACHE
    if _NC_CACHE is None:
        _NC_CACHE = build_kernel()
    return _NC_CACHE


def make_in_maps(inputs):
    """Split full inputs into 8 per-core input maps."""
    x = np.asarray(inputs["x"], np.float32)
    y = np.asarray(inputs["y"], np.float32)
    mask = np.asarray(inputs["mask"])
    shared = {}
    for n in WEIGHT_NAMES:
        shared[n] = np.ascontiguousarray(
            np.asarray(inputs[n], np.float32).astype(ml_dtypes.bfloat16))
    for n in VEC_NAMES:
        shared[n] = np.ascontiguousarray(np.asarray(inputs[n], np.float32))
    in_maps = []
    for c in range(8):
        b, qh = c // 2, c % 2
        m = dict(shared)
        m["xs"] = np.ascontiguousarray(x[b, qh * TQC:(qh + 1) * TQC, :])
        m["y"] = np.ascontiguousarray(y[b])
        m["mb"] = ((mask[b].astype(np.float32) - 1.0) * -MASK_NEG).astype(np.float32)
        in_maps.append(m)
    return in_maps


def assemble(results):
    outf = np.empty((B, TQ, D), np.float32)
    for c in range(8):
        b, qh = c // 2, c % 2
        outf[b, qh * TQC:(qh + 1) * TQC, :] = results[c]["out"]
    return outf


def kernel(**inputs) -> np.ndarray:
    nc = _get_nc()
    in_maps = make_in_maps(inputs)
    res = run_bass_kernel_spmd(nc, in_maps, list(range(8)))
    return assemble(res.results)


if __name__ == "__main__":
    nc = _get_nc()
    print("kernel built and compiled OK")



# revision 28
# speedup vs baseline: 1.0373x; 1.0373x over previous
"""CrossTransformer Trainium2 kernel.

Shapes (hardcoded): B=4, TQ=TK=1024, D=1024, H=16, DK=DV=64.
Sharding: 8 cores = 4 batches x 2 query-row halves. Each core computes
out[b, qs:qs+512, :] independently (k/v work duplicated across the pair
of cores sharing a batch; no collectives).

Weights, x and y are converted to bf16 on the host.

Layout strategy: all LN affine (gamma/beta) + ReLU are applied on the
Act engine during the PSUM->SBUF copy after the PE transpose, where the
per-feature gamma/beta become per-partition scalars. Softmax exp runs on
paired-head [128,2,512] PSUM tiles; emission of attention head-pairs is
interleaved with the v-projection and attn@v chunks so the PE never
starves while the Act engine streams the exps. Softmax denominators are
broadcast across partitions on the (otherwise idle) GPSIMD engine.
"""
import sys

for _p in ("/root/.axon_site", "/root/.axon_site/_ro/trn_rl_repo",
           "/root/.axon_site/_ro/pypackages", "/opt/trn_rl_repo"):
    if _p not in sys.path:
        sys.path.append(_p)

import numpy as np
import ml_dtypes
import concourse.bass as bass
from concourse import bacc
import concourse.tile as tile
import concourse.mybir as mybir
from concourse.masks import make_identity
from concourse.bass_utils import run_bass_kernel_spmd

F32 = mybir.dt.float32
BF = mybir.dt.bfloat16
F8 = mybir.dt.float8e4
DR = mybir.MatmulPerfMode.DoubleRow
AF = mybir.ActivationFunctionType
OP = mybir.AluOpType

B, TQ, TK, D = 4, 1024, 1024, 1024
H, DK, DV = 16, 64, 64
TQC = TQ // 2          # 512 query rows per core
NT = TQC // 128        # 4 q-row tiles
KD = D // 128          # 8 contraction chunks
MASK_NEG = -30000.0

WEIGHT_NAMES = ["q_w", "k_w", "v_w", "o_w", "l1_w", "l2_w"]
FP8_WEIGHTS = {"q_w", "k_w", "v_w", "o_w"}   # l1/l2 stay bf16 (error budget)
VEC_NAMES = ["q_b", "k_b", "v_b", "o_b", "l1_b", "l2_b",
             "ln1_g", "ln1_b", "ln2_g", "ln2_b",
             "mln1_g", "mln1_b", "mln2_g", "mln2_b"]
# vectors loaded p-major [128, 8] so element d lands on partition d%128,
# column d//128 (per-partition scalars for Act in feature-major layout)
PMAJ_NAMES = ["q_b", "k_b", "ln1_g", "ln1_b", "ln2_g", "ln2_b",
              "mln1_g", "mln1_b", "mln2_g", "mln2_b"]
# vectors broadcast to [128, 1024] (free-dim biases in token-major layout)
BCAST_NAMES = ["v_b", "o_b", "l1_b", "l2_b"]


def build_kernel(compile=True, repeat=1, stop_after=None, debug_dumps=False):
    nc = bacc.Bacc()
    xs = nc.dram_tensor("xs", (TQC, D), BF, kind="ExternalInput")
    y = nc.dram_tensor("y", (TK, D), BF, kind="ExternalInput")
    mb = nc.dram_tensor("mb", (TK,), F32, kind="ExternalInput")
    wd = {n: nc.dram_tensor(n, (D, D), F8 if n in FP8_WEIGHTS else BF,
                            kind="ExternalInput") for n in WEIGHT_NAMES}
    vd = {n: nc.dram_tensor(n, (D,), F32, kind="ExternalInput") for n in VEC_NAMES}
    out = nc.dram_tensor("out", (TQC, D), F32, kind="ExternalOutput")

    dbg = {}
    if debug_dumps:
        for nm, shape, dt_ in [
            ("d_x1T", (128, KD, TQC), F8), ("d_y1T", (128, KD, TK), F8),
            ("d_qT", (128, KD, TQC), BF), ("d_kT", (128, KD, TK), BF),
            ("d_vx", (128, KD, H, DV + 1), F8), ("d_e0", (128, 2, KD, TQC), F8),
            ("d_rb0", (64, TQC), F32), ("d_attnT", (128, KD, TQC), F8),
            ("d_xsb", (128, NT, D), BF), ("d_hsb", (128, NT, D), BF),
        ]:
            dbg[nm] = nc.dram_tensor(nm, shape, dt_, kind="ExternalOutput")

    with tile.TileContext(nc) as tc:
        for r in range(repeat):
            _emit(nc, tc, xs, y, mb, wd, vd, out, pfx=f"r{r}_", stop_after=stop_after,
                  dbg=dbg if r == 0 else {})
    if compile:
        nc.compile()
    return nc


def _emit(nc, tc, xs, y, mb, wd, vd, out, pfx="", stop_after=None, dbg=None):
    dbg = dbg or {}

    def dump(nm, src):
        if nm in dbg:
            nc.sync.dma_start(dbg[nm][:], src)
    from contextlib import ExitStack

    ctx = ExitStack()
    with ctx:
        persist = ctx.enter_context(tc.tile_pool(name=pfx + "persist", bufs=1))
        lnp = ctx.enter_context(tc.tile_pool(name=pfx + "lnp", bufs=3))
        bcast = ctx.enter_context(tc.tile_pool(name=pfx + "bcast", bufs=2))
        wts = ctx.enter_context(tc.tile_pool(name=pfx + "wts", bufs=2))
        ubuf = ctx.enter_context(tc.tile_pool(name=pfx + "ubuf", bufs=2))
        xtp = ctx.enter_context(tc.tile_pool(name=pfx + "xtp", bufs=1))

        # ---------------- constants ----------------
        ident = persist.tile([128, 128], BF, tag="ident", name=pfx + "ident")
        make_identity(nc, ident)
        eps_t = persist.tile([128, 1], F32, tag="eps", name=pfx + "eps")
        nc.vector.memset(eps_t[:], 1e-5)
        mb_sb = persist.tile([128, KD], F32, tag="mb_sb", name=pfx + "mb_sb")
        nc.sync.dma_start(mb_sb[:], mb.rearrange("(t p) -> p t", p=128))
        pm = {}
        for n in PMAJ_NAMES:
            t = persist.tile([128, KD], F32, tag=f"pm_{n}", name=pfx + f"pm_{n}")
            nc.sync.dma_start(t[:], vd[n].rearrange("(t p) -> p t", p=128))
            pm[n] = t

        def bcast_tile(name):
            t = bcast.tile([128, D], F32, tag="bc", name=pfx + f"bc_{name}")
            nc.sync.dma_start(t[:], vd[name][:].unsqueeze(0).partition_broadcast(128))
            return t

        def load_weight(name):
            fp8 = name in FP8_WEIGHTS
            wt = wts.tile([128, KD, D], F8 if fp8 else BF,
                          tag="w8" if fp8 else "w16", name=pfx + f"w_{name}")
            nc.sync.dma_start(wt[:], wd[name].rearrange("(ko p) n -> p ko n", p=128))
            return wt

        # LN stats for one [128, 1024] tile -> (mv [128,2], rstd [128,1])
        def ln_stats(src, key):
            stats = lnp.tile([128, 2, 6], F32, tag="stats", name=pfx + f"st_{key}")
            for i in range(2):
                nc.vector.bn_stats(stats[:, i, :], src[:, i * 512:(i + 1) * 512])
            mv = lnp.tile([128, 2], F32, tag="mv", name=pfx + f"mv_{key}")
            nc.vector.bn_aggr(mv[:], stats[:])
            std = lnp.tile([128, 1], F32, tag="std", name=pfx + f"sd_{key}")
            nc.scalar.activation(std[:], mv[:, 1:2], AF.Sqrt, bias=eps_t[:], scale=1.0)
            rstd = lnp.tile([128, 1], F32, tag="rstd", name=pfx + f"rs_{key}")
            nc.vector.reciprocal(rstd[:], std[:])
            return mv, rstd

        # Normalize+transpose+affine+relu a token-major [128, nt, 1024]
        # source into feature-major [128, KD, nt*128] bf16 dst.
        def ln_t(src, nt, g, b, dst, dst_col0, psT, key):
            u = ubuf.tile([128, nt, D], BF, tag="u", name=pfx + f"u_{key}")
            for t in range(nt):
                mv, rstd = ln_stats(src[:, t, :], f"{key}{t}")
                nc.vector.tensor_scalar(u[:, t, :], src[:, t, :], mv[:, 0:1],
                                        rstd[:], OP.subtract, OP.mult)
            for dt in range(KD):
                pt = psT.tile([128, nt * 128], BF, tag="tr", name=pfx + f"tr_{key}{dt}")
                for t in range(nt):
                    nc.tensor.transpose(pt[:, t * 128:(t + 1) * 128],
                                        u[:, t, dt * 128:(dt + 1) * 128], ident[:])
                nc.scalar.activation(
                    dst[:, dt, dst_col0:dst_col0 + nt * 128], pt[:], AF.Relu,
                    bias=b[:, dt:dt + 1], scale=g[:, dt:dt + 1])

        x_sb = persist.tile([128, NT, D], BF, tag="x_sb", name=pfx + "x_sb")
        qT = persist.tile([128, KD, TQC], BF, tag="qT", name=pfx + "qT")
        kT = persist.tile([128, KD, TK], BF, tag="kT", name=pfx + "kT")
        v_ext = persist.tile([128, KD, H, DV + 1], F8, tag="v_ext", name=pfx + "v_ext")
        attnT = persist.tile([128, KD, TQC], F8, tag="attnT", name=pfx + "attnT")
        x1T = xtp.tile([128, KD, TQC], F8, tag="xT8", name=pfx + "x1T")
        y1T = persist.tile([128, KD, TK], F8, tag="yT", name=pfx + "y1T")

        # ---------------- phase A: LN + relu + transpose ----------------
        with (
            tc.tile_pool(name=pfx + "psT", bufs=2, space="PSUM") as psT,
            tc.tile_pool(name=pfx + "yld", bufs=2) as yld,
        ):
            for t in range(NT):
                nc.sync.dma_start(x_sb[:, t, :],
                                  xs.rearrange("(t p) d -> p t d", p=128)[:, t, :])
            ln_t(x_sb, NT, pm["ln1_g"], pm["ln1_b"], x1T, 0, psT, "x")

            for hh in range(2):
                yl = yld.tile([128, NT, D], BF, tag="yl", name=pfx + f"yl{hh}")
                for t in range(NT):
                    nc.sync.dma_start(
                        yl[:, t, :],
                        y.rearrange("(t p) d -> p t d", p=128)[:, 4 * hh + t, :])
                ln_t(yl, NT, pm["ln2_g"], pm["ln2_b"], y1T, hh * 512, psT, f"y{hh}")

        dump("d_x1T", x1T[:])
        dump("d_y1T", y1T[:])
        if stop_after == "A":
            return

        # -------- phases C+D interleaved: projections + attention --------
        with (
            tc.tile_pool(name=pfx + "psP", bufs=2, space="PSUM") as psP,
            tc.tile_pool(name=pfx + "psE", bufs=2, space="PSUM") as psE,
            tc.tile_pool(name=pfx + "psV", bufs=2, space="PSUM") as psV,
            tc.tile_pool(name=pfx + "att", bufs=2) as att,
        ):
            wq = load_weight("q_w")
            wk = load_weight("k_w")
            bv = bcast_tile("v_b")
            # full-tile memset (not just the ones column): overlapping the
            # later v writes forces write-after-write ordering, avoiding a
            # byte-granularity RMW race between GPSIMD and DVE on HW
            nc.gpsimd.memset(v_ext[:], 1.0)

            def emit_q(m):
                pq = psP.tile([128, TQC], F32, tag="ps", name=pfx + f"pq{m}")
                for kc in range(0, KD, 2):
                    nc.tensor.matmul(pq[:], wq[:, kc:kc + 2, m * 128:(m + 1) * 128],
                                     x1T[:, kc:kc + 2, :], perf_mode=DR,
                                     start=(kc == 0), stop=(kc == KD - 2))
                nc.vector.tensor_scalar(qT[:, m, :], pq[:], pm["q_b"][:, m:m + 1],
                                        None, OP.add)

            def emit_k(m, hh):
                pk = psP.tile([128, 512], F32, tag="ps", name=pfx + f"pk{m}_{hh}")
                for kc in range(0, KD, 2):
                    nc.tensor.matmul(pk[:], wk[:, kc:kc + 2, m * 128:(m + 1) * 128],
                                     y1T[:, kc:kc + 2, hh * 512:(hh + 1) * 512],
                                     perf_mode=DR,
                                     start=(kc == 0), stop=(kc == KD - 2))
                nc.vector.tensor_scalar(kT[:, m, hh * 512:(hh + 1) * 512], pk[:],
                                        pm["k_b"][:, m:m + 1], None, OP.add)

            def emit_v(m, nt2, wv):
                pv = psP.tile([128, 512], F32, tag="ps", name=pfx + f"pv{m}_{nt2}")
                for kc in range(0, KD, 2):
                    nc.tensor.matmul(pv[:], y1T[:, kc:kc + 2, m * 128:(m + 1) * 128],
                                     wv[:, kc:kc + 2, nt2 * 512:(nt2 + 1) * 512],
                                     perf_mode=DR,
                                     start=(kc == 0), stop=(kc == KD - 2))
                nc.vector.tensor_tensor(
                    v_ext[:, m, nt2 * 8:(nt2 + 1) * 8, :DV],
                    pv.rearrange("p (h v) -> p h v", v=DV),
                    bv[:, nt2 * 512:(nt2 + 1) * 512].rearrange(
                        "p (h v) -> p h v", v=DV),
                    OP.add)

            e_sbs, rcs, pavs = {}, {}, {}

            def emit_logit(j, mt):
                if mt == 0:
                    e_sbs[j] = att.tile([128, 2, KD, TQC], F8, tag="e_sb",
                                        name=pfx + f"e{j}")
                ps = psE.tile([128, 2, TQC], F32, tag="pse", name=pfx + f"s{j}_{mt}")
                nc.tensor.matmul(ps[:, 0, :], kT[0:64, j, mt * 128:(mt + 1) * 128],
                                 qT[0:64, j, :], start=True, stop=True)
                nc.tensor.matmul(ps[:, 1, :], kT[64:128, j, mt * 128:(mt + 1) * 128],
                                 qT[64:128, j, :], start=True, stop=True)
                nc.scalar.activation(e_sbs[j][:, :, mt, :], ps[:], AF.Exp,
                                     bias=mb_sb[:, mt:mt + 1], scale=0.125)

            def emit_av(j, par):
                h = 2 * j + par
                if par == 0:
                    rcs[j] = att.tile([128, 2, TQC], F32, tag="rc", name=pfx + f"rc{j}")
                pav = psV.tile([128, TQC], F32, tag="pav", name=pfx + f"av{h}")
                pavs[(j, par)] = pav
                for kt in range(0, KD, 2):
                    nc.tensor.matmul(pav[:DV + 1, :], v_ext[:, kt:kt + 2, h, :],
                                     e_sbs[j][:, par, kt:kt + 2, :], perf_mode=DR,
                                     start=(kt == 0), stop=(kt == KD - 2))
                nc.vector.reciprocal(rcs[j][0:1, par, :], pav[DV:DV + 1, :])

            def emit_norm(j):
                # both broadcasts write partition-0-based rows: the GPSIMD
                # ucode does not support partition-offset outputs
                rb = att.tile([128, 2, TQC], F32, tag="rb", name=pfx + f"rb{j}")
                nc.gpsimd.partition_broadcast(rb[0:64, 0, :], rcs[j][0:1, 0, :])
                nc.gpsimd.partition_broadcast(rb[0:64, 1, :], rcs[j][0:1, 1, :])
                if j == 0:
                    dump("d_e0", e_sbs[0][:])
                    dump("d_rb0", rb[0:64, 0, :])
                nc.vector.tensor_tensor(attnT[0:DV, j, :], pavs[(j, 0)][:DV, :],
                                        rb[0:64, 0, :], OP.mult)
                nc.vector.tensor_tensor(attnT[DV:128, j, :], pavs[(j, 1)][:DV, :],
                                        rb[0:64, 1, :], OP.mult)
                del pavs[(j, 0)], pavs[(j, 1)], e_sbs[j], rcs[j]

            # --- schedule: keep PE busy while Act streams the exps ---
            for m in range(KD):
                emit_q(m)
            for m in range(KD):
                emit_k(m, 0)
                emit_k(m, 1)
            wv = load_weight("v_w")
            for m in range(6):
                emit_v(m, 0, wv)
            # fillers[j] = list of (slot_after_mt, thunk) for head pair j
            fillers = {j: [] for j in range(KD)}
            fillers[0] = [(2, lambda: emit_v(6, 0, wv)), (5, lambda: emit_v(7, 0, wv))]
            for j in range(1, KD):
                fl = [(1, lambda j=j: emit_av(j - 1, 0)),
                      (3, lambda j=j: emit_av(j - 1, 1)),
                      (4, lambda j=j: emit_norm(j - 1))]
                if j <= 4:
                    fl += [(5, lambda j=j: emit_v(2 * (j - 1), 1, wv)),
                           (6, lambda j=j: emit_v(2 * (j - 1) + 1, 1, wv))]
                fillers[j] = fl
            for j in range(KD):
                fl = list(fillers[j])
                for mt in range(KD):
                    emit_logit(j, mt)
                    for slot, thunk in fl:
                        if slot == mt:
                            thunk()
            emit_av(KD - 1, 0)
            emit_av(KD - 1, 1)
            emit_norm(KD - 1)
            dump("d_qT", qT[:])
            dump("d_kT", kT[:])
            dump("d_vx", v_ext[:])
            dump("d_attnT", attnT[:])

        if stop_after == "D":
            return

        # ---------------- phase E: o-proj + residual ----------------
        with tc.tile_pool(name=pfx + "psM", bufs=4, space="PSUM") as psM:
            wo = load_weight("o_w")
            bo = bcast_tile("o_b")
            for mt in range(NT):
                for nt2 in range(2):
                    po = psM.tile([128, 512], F32, tag="ps", name=pfx + f"po{mt}_{nt2}")
                    for kc in range(0, KD, 2):
                        nc.tensor.matmul(po[:],
                                         attnT[:, kc:kc + 2, mt * 128:(mt + 1) * 128],
                                         wo[:, kc:kc + 2, nt2 * 512:(nt2 + 1) * 512],
                                         perf_mode=DR,
                                         start=(kc == 0), stop=(kc == KD - 2))
                    sl = slice(nt2 * 512, (nt2 + 1) * 512)
                    nc.vector.tensor_tensor(x_sb[:, mt, sl], x_sb[:, mt, sl],
                                            po[:], OP.add)
                    nc.gpsimd.tensor_tensor(x_sb[:, mt, sl], x_sb[:, mt, sl],
                                            bo[:, sl], OP.add)

            dump("d_xsb", x_sb[:])
            if stop_after == "E":
                return

            # ---------------- phases F/G: MLP ----------------
            with (
                tc.tile_pool(name=pfx + "psT2", bufs=2, space="PSUM") as psT2,
                tc.tile_pool(name=pfx + "mlp", bufs=1) as mlp,
            ):
                z1T = xtp.tile([128, KD, TQC], BF, tag="xT", name=pfx + "z1T")
                ln_t(x_sb, NT, pm["mln1_g"], pm["mln1_b"], z1T, 0, psT2, "z1")
                w1 = load_weight("l1_w")
                bl1 = bcast_tile("l1_b")
                h_sb = mlp.tile([128, NT, D], BF, tag="h_sb", name=pfx + "h_sb")
                for mt in range(NT):
                    for nt2 in range(2):
                        ph = psM.tile([128, 512], F32, tag="ps",
                                      name=pfx + f"ph{mt}_{nt2}")
                        for kc in range(KD):
                            nc.tensor.matmul(ph[:], z1T[:, kc, mt * 128:(mt + 1) * 128],
                                             w1[:, kc, nt2 * 512:(nt2 + 1) * 512],
                                             start=(kc == 0), stop=(kc == KD - 1))
                        sl = slice(nt2 * 512, (nt2 + 1) * 512)
                        nc.vector.tensor_tensor(h_sb[:, mt, sl], ph[:], bl1[:, sl],
                                                OP.add)

                dump("d_hsb", h_sb[:])
                z2T = xtp.tile([128, KD, TQC], BF, tag="xT", name=pfx + "z2T")
                ln_t(h_sb, NT, pm["mln2_g"], pm["mln2_b"], z2T, 0, psT2, "z2")
                w2 = load_weight("l2_w")
                bl2 = bcast_tile("l2_b")
                out_r = out.rearrange("(t p) d -> p t d", p=128)
                for mt in range(NT):
                    o_sb = mlp.tile([128, D], F32, tag="o_sb", name=pfx + f"os{mt}")
                    for nt2 in range(2):
                        pf = psM.tile([128, 512], F32, tag="ps",
                                      name=pfx + f"pf{mt}_{nt2}")
                        for kc in range(KD):
                            nc.tensor.matmul(pf[:], z2T[:, kc, mt * 128:(mt + 1) * 128],
                                             w2[:, kc, nt2 * 512:(nt2 + 1) * 512],
                                             start=(kc == 0), stop=(kc == KD - 1))
                        sl = slice(nt2 * 512, (nt2 + 1) * 512)
                        nc.vector.tensor_tensor(o_sb[:, sl], pf[:], bl2[:, sl], OP.add)
                    nc.sync.dma_start(out_r[:, mt, :], o_sb[:])


_NC_CACHE = None


def _get_nc():
    global _NC_CACHE
    if _NC_CACHE is None:
        _NC_CACHE = build_kernel()
    return _NC_CACHE


def make_in_maps(inputs):
    """Split full inputs into 8 per-core input maps."""
    x = np.asarray(inputs["x"], np.float32)
    y = np.asarray(inputs["y"], np.float32)
    mask = np.asarray(inputs["mask"])
    shared = {}
    for n in WEIGHT_NAMES:
        dt_ = ml_dtypes.float8_e4m3fn if n in FP8_WEIGHTS else ml_dtypes.bfloat16
        shared[n] = np.ascontiguousarray(np.asarray(inputs[n], np.float32).astype(dt_))
    for n in VEC_NAMES:
        shared[n] = np.ascontiguousarray(np.asarray(inputs[n], np.float32))
    ybf = y.astype(ml_dtypes.bfloat16)
    xbf = x.astype(ml_dtypes.bfloat16)
    in_maps = []
    for c in range(8):
        b, qh = c // 2, c % 2
        m = dict(shared)
        m["xs"] = np.ascontiguousarray(xbf[b, qh * TQC:(qh + 1) * TQC, :])
        m["y"] = np.ascontiguousarray(ybf[b])
        m["mb"] = ((mask[b].astype(np.float32) - 1.0) * -MASK_NEG).astype(np.float32)
        in_maps.append(m)
    return in_maps


def assemble(results):
    outf = np.empty((B, TQ, D), np.float32)
    for c in range(8):
        b, qh = c // 2, c % 2
        outf[b, qh * TQC:(qh + 1) * TQC, :] = results[c]["out"]
    return outf


def kernel(**inputs) -> np.ndarray:
    nc = _get_nc()
    in_maps = make_in_maps(inputs)
    res = run_bass_kernel_spmd(nc, in_maps, list(range(8)))
    return assemble(res.results)


if __name__ == "__main__":
    nc = _get_nc()
    print("kernel built and compiled OK")
